# revision 1
# baseline (speedup 1.0000x reference)
"""Trainium2 Bass kernel for nn_MultiHeadCrossAttention (B=4, N=2048, C=256, H=4, d=64).

Sharding: 8 cores, core c -> (batch b = c//2, query-half qh = c%2).
Each core computes full 4-head cross-attention for its 1024-query slice of
its batch, plus the residuals and output projection. No collectives; the
host slices/transposes/casts inputs per core and concatenates the outputs.

With gamma == 0 (as produced by setup_inputs), the LAM channel-attention
block is exactly the identity, so:
    out = (t2_grad + q + attn_out) @ Wproj + bproj

v3 (bf16 + dual-engine exp): all matmul operands are bf16 (PSUM
accumulation stays fp32) — fp32/fp32r matmuls paid ~2x on TRN2 for the
projection/S streams, and bf16 halved Tensor-engine busy 101us -> 81us.
The softmax exp (8.4M elements/core, only ACT and DVE can read PSUM) is
split across both engines: ACT computes exact exp for the j=0 query
blocks (plus every third chunk's j=1), DVE computes a Schraudolph-style
bf16 exp (int16 round(S*A+B), bits reinterpreted as bf16) for the rest;
the ~5% sawtooth cancels between softmax numerator and denominator,
which use the same approximation per query.  Layout:
 - t1T/t2T: (C, keys/queries) bf16.  kT/qT = W^T @ tT via PE, heads
   pair-packed (tile m holds heads 2m, 2m+1 on partition halves);
   emission interleaves kT/v/qT per DMA chunk to absorb load latency.
 - v tiles per key chunk: (128, 4*65) bf16; head h cols = [1 | v_h] so
   the softmax denominator rides the xo matmul as output row 0.
 - S^T tiles (keys on partitions, queries free) pack both heads side by
   side in the free dim; the two K=64 matmuls hit PE row groups 0/64
   (throughput is PSUM-write-port bound at 128 values/cycle).
 - unnormalized xo^T accumulates in PSUM; normalization multiplies by a
   GpSimd-broadcast reciprocal into per-head [65, Q] tiles whose row 0
   (den*recip ~= 1) is killed by a zero row in the wp_h weights — except
   wp_h[0] row 0, which carries bproj so the bias rides the projection.
 - final projection: out = x^T.T @ Wproj with K-groups [t2T+qT (2x128),
   4 per-head xon (65)], split into pass A (pair 0, overlaps pair-1
   normalization) and pass B.
All engine ops keep in/out partition bases equal (DVE/ACT lanes are
partition-locked); cross-partition moves go through GpSimd broadcast.
"""

from contextlib import ExitStack

import numpy as np

import concourse.bass as bass
import concourse.mybir as mybir
import concourse.tile as tile
from concourse import bacc
from concourse.bass_utils import run_bass_kernel_spmd

B, N, C, H, D = 4, 2048, 256, 4, 64
NCORES = 8
Q = 1024  # queries per core
SCALE = float(D) ** -0.5
FP32 = mybir.dt.float32
BF16 = mybir.dt.bfloat16
I16 = mybir.dt.int16
AF = mybir.ActivationFunctionType
ALU = mybir.AluOpType

# Schraudolph bf16 exp: bitcast(int16(round(x * EXPA + EXPB))) ~= exp(x/8)
# (the softmax 1/sqrt(d) scale is folded into EXPA; C=16 centers the
# sawtooth error for round-to-nearest, measured on hardware)
EXPA = float(128.0 / np.log(2.0)) * SCALE
EXPB = 127.0 * 128.0 - 16.0

_CACHE = {}


def build_nc():
    nc = bacc.Bacc("TRN2", target_bir_lowering=False, debug=False,
                   num_devices=NCORES)
    MDT = BF16

    t1T_d = nc.dram_tensor("t1T", [C, N], MDT, kind="ExternalInput")
    t2T_d = nc.dram_tensor("t2T", [C, Q], MDT, kind="ExternalInput")
    wq_d = nc.dram_tensor("wq", [C, C], MDT, kind="ExternalInput")
    wk_d = nc.dram_tensor("wk", [C, C], MDT, kind="ExternalInput")
    wv_d = nc.dram_tensor("wv", [C, C], MDT, kind="ExternalInput")
    wp_d = nc.dram_tensor("wp", [C, C], MDT, kind="ExternalInput")
    bp_d = nc.dram_tensor("bp", [1, C], MDT, kind="ExternalInput")
    out_d = nc.dram_tensor("out", [Q, C], FP32, kind="ExternalOutput")

    with tile.TileContext(nc) as tc, ExitStack() as ctx:
        const = ctx.enter_context(tc.tile_pool(name="const", bufs=1))
        acts = ctx.enter_context(tc.tile_pool(name="acts", bufs=1))

        # ---- load inputs (critical path first: wk, then t1T chunks) ----
        w_sb = {}
        for name, dram in (("wk", wk_d), ("wq", wq_d), ("wv", wv_d),
                           ("wp", wp_d)):
            tiles = []
            for cc in range(2):
                t = const.tile([128, C], MDT, name=f"{name}{cc}",
                               tag=f"{name}{cc}")
                if name in ("wk", "wq"):
                    nc.sync.dma_start(out=t[:],
                                      in_=dram[cc * 128:(cc + 1) * 128, :])
                tiles.append(t)
            w_sb[name] = tiles

        t1T = [acts.tile([128, N], MDT, name=f"t1T{cc}", tag=f"t1T{cc}")
               for cc in range(2)]
        t2T = [acts.tile([128, Q], MDT, name=f"t2T{cc}", tag=f"t2T{cc}")
               for cc in range(2)]
        def load_t1(nn):
            for cc in range(2):
                nc.sync.dma_start(
                    out=t1T[cc][:, nn * 512:(nn + 1) * 512],
                    in_=t1T_d[cc * 128:(cc + 1) * 128, nn * 512:(nn + 1) * 512])

        def load_t2(nn):
            for cc in range(2):
                nc.sync.dma_start(
                    out=t2T[cc][:, nn * 512:(nn + 1) * 512],
                    in_=t2T_d[cc * 128:(cc + 1) * 128, nn * 512:(nn + 1) * 512])

        load_t1(0)
        for cc in range(2):
            nc.sync.dma_start(out=w_sb["wv"][cc][:],
                              in_=wv_d[cc * 128:(cc + 1) * 128, :])
        load_t2(0)
        load_t1(1)
        load_t2(1)
        for nn in range(2, N // 512):
            load_t1(nn)
        for cc in range(2):
            nc.sync.dma_start(out=w_sb["wp"][cc][:],
                              in_=wp_d[cc * 128:(cc + 1) * 128, :])

        # wp_h[h] row 0 multiplies xon[h] row 0 = den*recip ~= 1 in the
        # final projection; head 0 carries the bias there (bias * ~1),
        # heads 1-3 keep a zero row.
        wp_h = []
        for h in range(4):
            t = const.tile([65, C], MDT, name=f"wph{h}", tag=f"wph{h}")
            nc.gpsimd.memset(t[:], 0.0)
            nc.sync.dma_start(out=t[1:65, :], in_=wp_d[h * 64:(h + 1) * 64, :])
            if h == 0:
                nc.sync.dma_start(out=t[0:1, :], in_=bp_d[:])
            wp_h.append(t)

        # ---- phase 1: projections kT, qT, v ----
        kT = [acts.tile([128, N], MDT, name=f"kT{m}", tag=f"kT{m}")
              for m in range(2)]
        qT = [acts.tile([128, Q], MDT, name=f"qT{m}", tag=f"qT{m}")
              for m in range(2)]
        v_sb = []
        for kc in range(16):
            t = acts.tile([128, 4 * 65], MDT, name=f"v{kc}", tag=f"v{kc}")
            # pre-fill with 1.0: cols 64/129/194/259 stay as the softmax
            # denominator "ones" columns; the rest is overwritten with v
            nc.gpsimd.memset(t[:], 1.0)
            v_sb.append(t)

        # attention pools (opened before projections so the first S/exp
        # pairs can be hoisted into the projection phase)
        xT = [acts.tile([128, Q], MDT, name=f"xT{m}", tag=f"xT{m}")
              for m in range(2)]
        # normalized attention outputs, one [65, Q] tile per head; row 0
        # holds den*recip ~= 1 and is killed by the zero row in wp_h
        xon = [acts.tile([65, Q], MDT, name=f"xon{h}", tag=f"xon{h}")
               for h in range(4)]
        attn_ctx = ExitStack()
        spool = attn_ctx.enter_context(
            tc.tile_pool(name="spsum", bufs=1, space="PSUM"))
        ppool2 = ctx.enter_context(tc.tile_pool(name="pexp", bufs=8))
        npool = ctx.enter_context(tc.tile_pool(name="norm", bufs=2))
        hoisted = []

        def emit_s_exp(m, kc):
            s_ts = []
            for j in range(Q // 512):
                s_t = spool.tile([128, Q], FP32, name=f"sq{j}", tag=f"sq{j}")
                for hh in range(2):
                    base = hh * 64
                    nc.tensor.matmul(
                        s_t[:, hh * 512:(hh + 1) * 512],
                        lhsT=kT[m][base:base + 64, kc * 128:(kc + 1) * 128],
                        rhs=qT[m][base:base + 64, j * 512:(j + 1) * 512],
                        start=True, stop=True)
                s_ts.append(s_t)
            # exp split across engines (GpSimd cannot read PSUM, so the
            # only softmax-capable engines are ACT and DVE): queries of
            # j=0 get the exact ACT exp (bf16 out, to match the bf16 v
            # weights in the xo matmul); queries of j=1 get
            # a Schraudolph-style bf16 exp on DVE (int16 = round(S*A+B),
            # bits reinterpreted as bf16; the ~5% sawtooth cancels
            # between softmax numerator and denominator, which use the
            # same approximation per query).
            pe0 = ppool2.tile([128, Q], MDT, name="pexp0", tag="pexp0")
            nc.scalar.activation(pe0[:], s_ts[0][:], AF.Exp, scale=SCALE)
            if kc % 3 == 0:
                # load-balance: ACT takes both tiles for some chunks
                pe1 = ppool2.tile([128, Q], MDT, name="pexp1", tag="pexp1")
                nc.scalar.activation(pe1[:], s_ts[1][:], AF.Exp, scale=SCALE)
            else:
                pe1 = ppool2.tile([128, Q], I16, name="pexp1", tag="pexp1")
                nc.vector.tensor_scalar(pe1[:], s_ts[1][:], EXPA, EXPB,
                                        op0=ALU.mult, op1=ALU.add)
            return [pe0, pe1]

        with tc.tile_pool(name="ppsum", bufs=2, space="PSUM") as ppool:
            for m in range(2):
                for nn in range(N // 512):
                    ps = ppool.tile([128, 512], FP32, name="p", tag="p")
                    for cc in range(2):
                        nc.tensor.matmul(
                            ps[:],
                            lhsT=w_sb["wk"][cc][:, m * 128:(m + 1) * 128],
                            rhs=t1T[cc][:, nn * 512:(nn + 1) * 512],
                            start=(cc == 0), stop=(cc == 1))
                    nc.vector.tensor_copy(kT[m][:, nn * 512:(nn + 1) * 512],
                                          ps[:])
            for m in range(2):
                for nn in range(Q // 512):
                    ps = ppool.tile([128, 512], FP32, name="p", tag="p")
                    for cc in range(2):
                        nc.tensor.matmul(
                            ps[:],
                            lhsT=w_sb["wq"][cc][:, m * 128:(m + 1) * 128],
                            rhs=t2T[cc][:, nn * 512:(nn + 1) * 512],
                            start=(cc == 0), stop=(cc == 1))
                    nc.vector.tensor_copy(qT[m][:, nn * 512:(nn + 1) * 512],
                                          ps[:])
            hoisted.append(emit_s_exp(0, 0))
            hoisted.append(emit_s_exp(0, 1))
            hoisted.append(emit_s_exp(0, 2))
            hoisted.append(emit_s_exp(0, 3))
            for kc in range(16):
                ps = ppool.tile([128, C], FP32, name="p", tag="p")
                for cc in range(2):
                    nc.tensor.matmul(
                        ps[:],
                        lhsT=t1T[cc][:, kc * 128:(kc + 1) * 128],
                        rhs=w_sb["wv"][cc][:],
                        start=(cc == 0), stop=(cc == 1))
                v3 = v_sb[kc][:].rearrange("p (h e) -> p h e", e=65)
                nc.vector.tensor_copy(
                    v3[:, :, 1:65],
                    ps[:].rearrange("p (h e) -> p h e", e=64))

        def emit_xo(m, kc, xo_ps, pes):
            for j in range(Q // 512):
                for hh in range(2):
                    h = 2 * m + hh
                    rhs = pes[j][:, hh * 512:(hh + 1) * 512]
                    if rhs.dtype == I16:
                        rhs = rhs.bitcast(MDT)
                    nc.tensor.matmul(
                        xo_ps[hh][0:65, j * 512:(j + 1) * 512],
                        lhsT=v_sb[kc][:, h * 65:(h + 1) * 65],
                        rhs=rhs,
                        start=(kc == 0), stop=(kc == 15))

        xopool = attn_ctx.enter_context(
            tc.tile_pool(name="xopsum", bufs=1, space="PSUM"))

        osb = ctx.enter_context(tc.tile_pool(name="osb", bufs=3))
        partial_pool = ctx.enter_context(tc.tile_pool(name="opart", bufs=1))
        partials = []

        for m in range(2):  # head pair (2m, 2m+1)
            nc.gpsimd.tensor_add(xT[m][:], t2T[m][:], qT[m][:])

            xo_ps = [xopool.tile([65, Q], FP32, name=f"xo{hh}", tag=f"xo{hh}")
                     for hh in range(2)]

            pending = list(hoisted)
            hoisted = []
            for kc in range(16):
                if kc >= len(pending):
                    pending.append(emit_s_exp(m, kc))
                if kc + 1 < 16 and kc + 1 >= len(pending):
                    # S/exp for the next chunk goes out before this chunk's
                    # xo so the PE refills ACT's pipeline first
                    pending.append(emit_s_exp(m, kc + 1))
                emit_xo(m, kc, xo_ps, pending[kc])
            del pending

            if m == 0:
                # keep ACT fed across the pair boundary: next pair's first
                # S/exp pairs go out before this pair's normalization chain
                hoisted.append(emit_s_exp(1, 0))
                hoisted.append(emit_s_exp(1, 1))
                hoisted.append(emit_s_exp(1, 2))
            else:
                # final-projection pass A: out_partial = (t2+q) @ Wproj
                # + pair-0 heads (+ bias via wp_h[0] row 0). Depends only
                # on pair-0 results, so the PE runs it while DVE/GpSimd
                # normalize pair 1 below.
                for mq in range(Q // 128):
                    ps = spool.tile([128, C], FP32, name="oA",
                                    tag=f"sq{mq % 2}")
                    for cc in range(2):
                        nc.tensor.matmul(
                            ps[:],
                            lhsT=xT[cc][:, mq * 128:(mq + 1) * 128],
                            rhs=w_sb["wp"][cc][:],
                            start=(cc == 0), stop=False)
                    for h in range(2):
                        nc.tensor.matmul(
                            ps[:],
                            lhsT=xon[h][:, mq * 128:(mq + 1) * 128],
                            rhs=wp_h[h][:],
                            start=False, stop=(h == 1))
                    part = partial_pool.tile([128, C], FP32, name="part",
                                             tag=f"part{mq}")
                    nc.vector.tensor_copy(part[:], ps[:])
                    partials.append(part)

            # normalize: row 0 of xo_ps[hh] = sum_k exp(S)
            for hh in range(2):
                recip = npool.tile([1, Q], FP32, name=f"recip{hh}",
                                   tag=f"recip{hh}")
                nc.vector.reciprocal_approx_fast(recip[:, :],
                                                 xo_ps[hh][0:1, :])
                bc_sb = npool.tile([65, Q], FP32, name=f"bc{hh}",
                                   tag=f"bc{hh}")
                nc.gpsimd.partition_broadcast(bc_sb[:], recip[:])
                nc.vector.tensor_mul(xon[2 * m + hh][:], xo_ps[hh][0:65, :],
                                     bc_sb[:])

        # final-projection pass B: add pair-1 heads to the partials
        for mq in range(Q // 128):
            ps = spool.tile([128, C], FP32, name="oB", tag=f"sq{mq % 2}")
            for h in range(2, 4):
                nc.tensor.matmul(
                    ps[:],
                    lhsT=xon[h][:, mq * 128:(mq + 1) * 128],
                    rhs=wp_h[h][:],
                    start=(h == 2), stop=(h == 3))
            o_sb = osb.tile([128, C], FP32, name="o", tag="o")
            nc.vector.tensor_add(o_sb[:], ps[:], partials[mq][:])
            nc.sync.dma_start(out=out_d[mq * 128:(mq + 1) * 128, :],
                              in_=o_sb[:])

        attn_ctx.close()

    nc.finalize()
    return nc


def _get_nc():
    if "nc" not in _CACHE:
        _CACHE["nc"] = build_nc()
    return _CACHE["nc"]


def _bf16(a):
    import ml_dtypes

    return np.ascontiguousarray(a.astype(ml_dtypes.bfloat16))


def make_in_maps(t2_grad, t1, Wq, Wkv, Wproj, bproj):
    t2 = np.asarray(t2_grad, dtype=np.float32)
    t1 = np.asarray(t1, dtype=np.float32)
    wq = _bf16(np.asarray(Wq, dtype=np.float32))
    wk = _bf16(np.ascontiguousarray(Wkv[:, :C], dtype=np.float32))
    wv = _bf16(np.ascontiguousarray(Wkv[:, C:], dtype=np.float32))
    wp = _bf16(np.asarray(Wproj, dtype=np.float32))
    bp = _bf16(np.asarray(bproj, dtype=np.float32).reshape(1, C))
    in_maps = []
    for c in range(NCORES):
        b, qh = c // 2, c % 2
        in_maps.append({
            "t1T": _bf16(t1[b].T),
            "t2T": _bf16(t2[b].T[:, qh * Q:(qh + 1) * Q]),
            "wq": wq, "wk": wk, "wv": wv, "wp": wp, "bp": bp,
        })
    return in_maps


def kernel(t2_grad, t1, Wq, Wkv, Wproj, bproj, gamma, _trace=False,
           _use_fp32r=True):
    gamma = np.asarray(gamma)
    if float(np.abs(gamma).max()) != 0.0:
        # LAM block is only the identity for gamma == 0; fall back to a
        # host reference for the general case (not exercised by the
        # reference setup_inputs, which fixes gamma = 0).
        return _host_reference(t2_grad, t1, Wq, Wkv, Wproj, bproj, gamma)

    nc = _get_nc()
    in_maps = make_in_maps(t2_grad, t1, Wq, Wkv, Wproj, bproj)
    res = run_bass_kernel_spmd(nc, in_maps, list(range(NCORES)), trace=_trace)
    out = np.empty((B, N, C), dtype=np.float32)
    for c in range(NCORES):
        b, qh = c // 2, c % 2
        out[b, qh * Q:(qh + 1) * Q, :] = res.results[c]["out"]
    if _trace:
        _CACHE["last_result"] = res
    return out


def _host_reference(t2_grad, t1, Wq, Wkv, Wproj, bproj, gamma):
    t2 = np.asarray(t2_grad, dtype=np.float64)
    t1 = np.asarray(t1, dtype=np.float64)
    Wq = np.asarray(Wq, dtype=np.float64)
    Wkv = np.asarray(Wkv, dtype=np.float64)
    Wproj = np.asarray(Wproj, dtype=np.float64)
    bproj = np.asarray(bproj, dtype=np.float64)
    g = float(np.asarray(gamma).reshape(-1)[0])
    q = (t2 @ Wq).reshape(B, N, H, D).transpose(0, 2, 1, 3)
    kv = (t1 @ Wkv).reshape(B, N, 2, H, D).transpose(2, 0, 3, 1, 4)
    k, v = kv[0], kv[1]
    s = np.einsum('bhnd,bhmd->bhnm', q, k) * SCALE
    s = s - s.max(axis=-1, keepdims=True)
    p = np.exp(s)
    p /= p.sum(axis=-1, keepdims=True)
    x = np.einsum('bhnm,bhmd->bhnd', p, v)
    xp = x.transpose(0, 3, 1, 2).reshape(B, D, H * N)
    energy = xp @ xp.transpose(0, 2, 1)
    energy = energy - energy.max(axis=-1, keepdims=True)
    att = np.exp(energy)
    att /= att.sum(axis=-1, keepdims=True)
    lam_out = (att @ xp).reshape(B, D, H, N)
    lam_out = g * lam_out + xp.reshape(B, D, H, N)
    x = lam_out.transpose(0, 2, 3, 1)
    xo = x.transpose(0, 2, 1, 3).reshape(B, N, C) \
        + q.transpose(0, 2, 1, 3).reshape(B, N, C)
    return ((t2 + xo) @ Wproj + bproj).astype(np.float32)



# revision 5
# speedup vs baseline: 1.0223x; 1.0223x over previous
"""Trainium2 Bass kernel for nn_MultiHeadCrossAttention (B=4, N=2048, C=256, H=4, d=64).

Sharding: 8 cores, core c -> (batch b = c//2, query-half qh = c%2).
Each core computes full 4-head cross-attention for its 1024-query slice of
its batch, plus the residuals and output projection. No collectives; the
host slices/transposes/casts inputs per core and concatenates the outputs.

With gamma == 0 (as produced by setup_inputs), the LAM channel-attention
block is exactly the identity, so:
    out = (t2_grad + q + attn_out) @ Wproj + bproj

v3 (bf16 + dual-engine exp): all matmul operands are bf16 (PSUM
accumulation stays fp32) — fp32/fp32r matmuls paid ~2x on TRN2 for the
projection/S streams, and bf16 halved Tensor-engine busy 101us -> 81us.
The softmax exp (8.4M elements/core, only ACT and DVE can read PSUM) is
split across both engines: ACT computes exact exp for the j=0 query
blocks (plus every third chunk's j=1), DVE computes a Schraudolph-style
bf16 exp (int16 round(S*A+B), bits reinterpreted as bf16) for the rest;
the ~5% sawtooth cancels between softmax numerator and denominator,
which use the same approximation per query.  Layout:
 - t1T/t2T: (C, keys/queries) bf16.  kT/qT = W^T @ tT via PE, heads
   pair-packed (tile m holds heads 2m, 2m+1 on partition halves);
   emission interleaves kT/v/qT per DMA chunk to absorb load latency.
 - v tiles per key chunk: (128, 4*65) bf16; head h cols = [1 | v_h] so
   the softmax denominator rides the xo matmul as output row 0.
 - S^T tiles (keys on partitions, queries free) pack both heads side by
   side in the free dim; the two K=64 matmuls hit PE row groups 0/64
   (throughput is PSUM-write-port bound at 128 values/cycle).
 - unnormalized xo^T accumulates in PSUM; normalization multiplies by a
   GpSimd-broadcast reciprocal into per-head [65, Q] tiles whose row 0
   (den*recip ~= 1) is killed by a zero row in the wp_h weights — except
   wp_h[0] row 0, which carries bproj so the bias rides the projection.
 - final projection: out = x^T.T @ Wproj with K-groups [t2T+qT (2x128),
   4 per-head xon (65)], split into pass A (pair 0, overlaps pair-1
   normalization) and pass B.
All engine ops keep in/out partition bases equal (DVE/ACT lanes are
partition-locked); cross-partition moves go through GpSimd broadcast.
"""

from contextlib import ExitStack

import numpy as np

import concourse.bass as bass
import concourse.mybir as mybir
import concourse.tile as tile
from concourse import bacc
from concourse.bass_utils import run_bass_kernel_spmd

B, N, C, H, D = 4, 2048, 256, 4, 64
NCORES = 8
Q = 1024  # queries per core
SCALE = float(D) ** -0.5
FP32 = mybir.dt.float32
BF16 = mybir.dt.bfloat16
I16 = mybir.dt.int16
AF = mybir.ActivationFunctionType
ALU = mybir.AluOpType

# Schraudolph bf16 exp: bitcast(int16(round(x * EXPA + EXPB))) ~= exp(x/8)
# (the softmax 1/sqrt(d) scale is folded into EXPA; C=16 centers the
# sawtooth error for round-to-nearest, measured on hardware)
EXPA = float(128.0 / np.log(2.0)) * SCALE
EXPB = 127.0 * 128.0 - 16.0

_CACHE = {}


def build_nc():
    nc = bacc.Bacc("TRN2", target_bir_lowering=False, debug=False,
                   num_devices=NCORES)
    MDT = BF16

    t1T_d = nc.dram_tensor("t1T", [C, N], MDT, kind="ExternalInput")
    t2T_d = nc.dram_tensor("t2T", [C, Q], MDT, kind="ExternalInput")
    wq_d = nc.dram_tensor("wq", [C, C], MDT, kind="ExternalInput")
    wk_d = nc.dram_tensor("wk", [C, C], MDT, kind="ExternalInput")
    wv_d = nc.dram_tensor("wv", [C, C], MDT, kind="ExternalInput")
    wp_d = nc.dram_tensor("wp", [C, C], MDT, kind="ExternalInput")
    bp_d = nc.dram_tensor("bp", [1, C], MDT, kind="ExternalInput")
    out_d = nc.dram_tensor("out", [Q, C], FP32, kind="ExternalOutput")

    with tile.TileContext(nc) as tc, ExitStack() as ctx:
        const = ctx.enter_context(tc.tile_pool(name="const", bufs=1))
        acts = ctx.enter_context(tc.tile_pool(name="acts", bufs=1))

        # ---- load inputs (critical path first: wk, then t1T chunks) ----
        # DMAs are spread across BOTH HWDGE queues (sync=SP, scalar=ACT);
        # a single queue serializes at ~600ns per 128x512 chunk and left
        # the PE idle until 12.7us.  cc=0 chunks ride SP, cc=1 rides ACT.
        dmae = [nc.sync, nc.scalar]
        w_sb = {}
        for name, dram in (("wk", wk_d), ("wq", wq_d), ("wv", wv_d),
                           ("wp", wp_d)):
            tiles = []
            for cc in range(2):
                t = const.tile([128, C], MDT, name=f"{name}{cc}",
                               tag=f"{name}{cc}")
                if name in ("wk", "wq"):
                    dmae[cc].dma_start(out=t[:],
                                       in_=dram[cc * 128:(cc + 1) * 128, :])
                tiles.append(t)
            w_sb[name] = tiles

        t1T = [acts.tile([128, N], MDT, name=f"t1T{cc}", tag=f"t1T{cc}")
               for cc in range(2)]
        t2T = [acts.tile([128, Q], MDT, name=f"t2T{cc}", tag=f"t2T{cc}")
               for cc in range(2)]
        def load_t1(nn):
            for cc in range(2):
                dmae[cc].dma_start(
                    out=t1T[cc][:, nn * 512:(nn + 1) * 512],
                    in_=t1T_d[cc * 128:(cc + 1) * 128, nn * 512:(nn + 1) * 512])

        def load_t2(nn):
            for cc in range(2):
                dmae[cc].dma_start(
                    out=t2T[cc][:, nn * 512:(nn + 1) * 512],
                    in_=t2T_d[cc * 128:(cc + 1) * 128, nn * 512:(nn + 1) * 512])

        load_t1(0)
        for cc in range(2):
            dmae[cc].dma_start(out=w_sb["wv"][cc][:],
                               in_=wv_d[cc * 128:(cc + 1) * 128, :])
        load_t2(0)
        load_t1(1)
        load_t2(1)
        for nn in range(2, N // 512):
            load_t1(nn)
        for cc in range(2):
            dmae[cc].dma_start(out=w_sb["wp"][cc][:],
                               in_=wp_d[cc * 128:(cc + 1) * 128, :])

        # wp_h[h] row 0 multiplies xon[h] row 0 = den*recip ~= 1 in the
        # final projection; head 0 carries the bias there (bias * ~1),
        # heads 1-3 keep a zero row.
        wp_h = []
        for h in range(4):
            t = const.tile([65, C], MDT, name=f"wph{h}", tag=f"wph{h}")
            nc.gpsimd.memset(t[:], 0.0)
            dmae[h % 2].dma_start(out=t[1:65, :],
                                  in_=wp_d[h * 64:(h + 1) * 64, :])
            if h == 0:
                nc.sync.dma_start(out=t[0:1, :], in_=bp_d[:])
            wp_h.append(t)

        # ---- phase 1: projections kT, qT, v ----
        kT = [acts.tile([128, N], MDT, name=f"kT{m}", tag=f"kT{m}")
              for m in range(2)]
        qT = [acts.tile([128, Q], MDT, name=f"qT{m}", tag=f"qT{m}")
              for m in range(2)]
        v_sb = []
        for kc in range(16):
            t = acts.tile([128, 4 * 65], MDT, name=f"v{kc}", tag=f"v{kc}")
            # pre-fill with 1.0: cols 64/129/194/259 stay as the softmax
            # denominator "ones" columns; the rest is overwritten with v
            nc.gpsimd.memset(t[:], 1.0)
            v_sb.append(t)

        # attention pools (opened before projections so the first S/exp
        # pairs can be hoisted into the projection phase)
        xT = [acts.tile([128, Q], MDT, name=f"xT{m}", tag=f"xT{m}")
              for m in range(2)]
        # normalized attention outputs, one [65, Q] tile per head; row 0
        # holds den*recip ~= 1 and is killed by the zero row in wp_h
        xon = [acts.tile([65, Q], MDT, name=f"xon{h}", tag=f"xon{h}")
               for h in range(4)]
        attn_ctx = ExitStack()
        spool = attn_ctx.enter_context(
            tc.tile_pool(name="spsum", bufs=1, space="PSUM"))
        ppool2 = ctx.enter_context(tc.tile_pool(name="pexp", bufs=8))
        npool = ctx.enter_context(tc.tile_pool(name="norm", bufs=2))
        hoisted = []

        def emit_s_exp(m, kc):
            s_ts = []
            for j in range(Q // 512):
                s_t = spool.tile([128, Q], FP32, name=f"sq{j}", tag=f"sq{j}")
                for hh in range(2):
                    base = hh * 64
                    nc.tensor.matmul(
                        s_t[:, hh * 512:(hh + 1) * 512],
                        lhsT=kT[m][base:base + 64, kc * 128:(kc + 1) * 128],
                        rhs=qT[m][base:base + 64, j * 512:(j + 1) * 512],
                        start=True, stop=True)
                s_ts.append(s_t)
            # exp split across engines (GpSimd cannot read PSUM, so the
            # only softmax-capable engines are ACT and DVE): queries of
            # j=0 get the exact ACT exp (bf16 out, to match the bf16 v
            # weights in the xo matmul); queries of j=1 get
            # a Schraudolph-style bf16 exp on DVE (int16 = round(S*A+B),
            # bits reinterpreted as bf16; the ~5% sawtooth cancels
            # between softmax numerator and denominator, which use the
            # same approximation per query).
            pe0 = ppool2.tile([128, Q], MDT, name="pexp0", tag="pexp0")
            nc.scalar.activation(pe0[:], s_ts[0][:], AF.Exp, scale=SCALE)
            if kc % 3 == 0:
                # load-balance: ACT takes both tiles for some chunks
                pe1 = ppool2.tile([128, Q], MDT, name="pexp1", tag="pexp1")
                nc.scalar.activation(pe1[:], s_ts[1][:], AF.Exp, scale=SCALE)
            else:
                pe1 = ppool2.tile([128, Q], I16, name="pexp1", tag="pexp1")
                nc.vector.tensor_scalar(pe1[:], s_ts[1][:], EXPA, EXPB,
                                        op0=ALU.mult, op1=ALU.add)
            return [pe0, pe1]

        # PSUM evacuation casts alternate DVE / ACT: both engines are
        # otherwise idle in this phase and each copy is ~0.5us.
        def evac(i, out, in_):
            if i % 3 != 2:
                nc.vector.tensor_copy(out, in_)
            else:
                nc.scalar.copy(out, in_)

        with tc.tile_pool(name="ppsum", bufs=2, space="PSUM") as ppool:
            for m in range(2):
                for nn in range(N // 512):
                    ps = ppool.tile([128, 512], FP32, name="p", tag="p")
                    for cc in range(2):
                        nc.tensor.matmul(
                            ps[:],
                            lhsT=w_sb["wk"][cc][:, m * 128:(m + 1) * 128],
                            rhs=t1T[cc][:, nn * 512:(nn + 1) * 512],
                            start=(cc == 0), stop=(cc == 1))
                    evac(m * 4 + nn, kT[m][:, nn * 512:(nn + 1) * 512],
                         ps[:])
            for m in range(2):
                for nn in range(Q // 512):
                    ps = ppool.tile([128, 512], FP32, name="p", tag="p")
                    for cc in range(2):
                        nc.tensor.matmul(
                            ps[:],
                            lhsT=w_sb["wq"][cc][:, m * 128:(m + 1) * 128],
                            rhs=t2T[cc][:, nn * 512:(nn + 1) * 512],
                            start=(cc == 0), stop=(cc == 1))
                    evac(m * 2 + nn, qT[m][:, nn * 512:(nn + 1) * 512],
                         ps[:])
            hoisted.append(emit_s_exp(0, 0))
            hoisted.append(emit_s_exp(0, 1))
            hoisted.append(emit_s_exp(0, 2))
            hoisted.append(emit_s_exp(0, 3))
            for kc in range(16):
                ps = ppool.tile([128, C], FP32, name="p", tag="p")
                for cc in range(2):
                    nc.tensor.matmul(
                        ps[:],
                        lhsT=t1T[cc][:, kc * 128:(kc + 1) * 128],
                        rhs=w_sb["wv"][cc][:],
                        start=(cc == 0), stop=(cc == 1))
                v3 = v_sb[kc][:].rearrange("p (h e) -> p h e", e=65)
                evac(kc, v3[:, :, 1:65],
                     ps[:].rearrange("p (h e) -> p h e", e=64))

        def emit_xo(m, kc, xo_ps, pes):
            for j in range(Q // 512):
                for hh in range(2):
                    h = 2 * m + hh
                    rhs = pes[j][:, hh * 512:(hh + 1) * 512]
                    if rhs.dtype == I16:
                        rhs = rhs.bitcast(MDT)
                    nc.tensor.matmul(
                        xo_ps[hh][0:65, j * 512:(j + 1) * 512],
                        lhsT=v_sb[kc][:, h * 65:(h + 1) * 65],
                        rhs=rhs,
                        start=(kc == 0), stop=(kc == 15))

        xopool = attn_ctx.enter_context(
            tc.tile_pool(name="xopsum", bufs=1, space="PSUM"))

        osb = ctx.enter_context(tc.tile_pool(name="osb", bufs=3))
        partial_pool = ctx.enter_context(tc.tile_pool(name="opart", bufs=1))
        partials = []

        for m in range(2):  # head pair (2m, 2m+1)
            nc.gpsimd.tensor_add(xT[m][:], t2T[m][:], qT[m][:])

            xo_ps = [xopool.tile([65, Q], FP32, name=f"xo{hh}", tag=f"xo{hh}")
                     for hh in range(2)]

            pending = list(hoisted)
            hoisted = []
            for kc in range(16):
                if kc >= len(pending):
                    pending.append(emit_s_exp(m, kc))
                if kc + 1 < 16 and kc + 1 >= len(pending):
                    # S/exp for the next chunk goes out before this chunk's
                    # xo so the PE refills ACT's pipeline first
                    pending.append(emit_s_exp(m, kc + 1))
                emit_xo(m, kc, xo_ps, pending[kc])
            del pending

            if m == 0:
                # keep ACT fed across the pair boundary: next pair's first
                # S/exp pairs go out before this pair's normalization chain
                hoisted.append(emit_s_exp(1, 0))
                hoisted.append(emit_s_exp(1, 1))
                hoisted.append(emit_s_exp(1, 2))
            else:
                # final-projection pass A: out_partial = (t2+q) @ Wproj
                # + pair-0 heads (+ bias via wp_h[0] row 0). Depends only
                # on pair-0 results, so the PE runs it while DVE/GpSimd
                # normalize pair 1 below.
                for mq in range(Q // 128):
                    ps = spool.tile([128, C], FP32, name="oA",
                                    tag=f"sq{mq % 2}")
                    for cc in range(2):
                        nc.tensor.matmul(
                            ps[:],
                            lhsT=xT[cc][:, mq * 128:(mq + 1) * 128],
                            rhs=w_sb["wp"][cc][:],
                            start=(cc == 0), stop=False)
                    for h in range(2):
                        nc.tensor.matmul(
                            ps[:],
                            lhsT=xon[h][:, mq * 128:(mq + 1) * 128],
                            rhs=wp_h[h][:],
                            start=False, stop=(h == 1))
                    part = partial_pool.tile([128, C], FP32, name="part",
                                             tag=f"part{mq}")
                    nc.vector.tensor_copy(part[:], ps[:])
                    partials.append(part)

            # normalize: row 0 of xo_ps[hh] = sum_k exp(S)
            for hh in range(2):
                recip = npool.tile([1, Q], FP32, name=f"recip{hh}",
                                   tag=f"recip{hh}")
                nc.vector.reciprocal_approx_fast(recip[:, :],
                                                 xo_ps[hh][0:1, :])
                bc_sb = npool.tile([65, Q], FP32, name=f"bc{hh}",
                                   tag=f"bc{hh}")
                nc.gpsimd.partition_broadcast(bc_sb[:], recip[:])
                nc.vector.tensor_mul(xon[2 * m + hh][:], xo_ps[hh][0:65, :],
                                     bc_sb[:])

        # final-projection pass B: add pair-1 heads to the partials
        for mq in range(Q // 128):
            ps = spool.tile([128, C], FP32, name="oB", tag=f"sq{mq % 2}")
            for h in range(2, 4):
                nc.tensor.matmul(
                    ps[:],
                    lhsT=xon[h][:, mq * 128:(mq + 1) * 128],
                    rhs=wp_h[h][:],
                    start=(h == 2), stop=(h == 3))
            o_sb = osb.tile([128, C], FP32, name="o", tag="o")
            nc.vector.tensor_add(o_sb[:], ps[:], partials[mq][:])
            dmae[mq % 2].dma_start(out=out_d[mq * 128:(mq + 1) * 128, :],
                                   in_=o_sb[:])

        attn_ctx.close()

    nc.finalize()
    return nc


def _get_nc():
    if "nc" not in _CACHE:
        _CACHE["nc"] = build_nc()
    return _CACHE["nc"]


def _bf16(a):
    import ml_dtypes

    return np.ascontiguousarray(a.astype(ml_dtypes.bfloat16))


def make_in_maps(t2_grad, t1, Wq, Wkv, Wproj, bproj):
    t2 = np.asarray(t2_grad, dtype=np.float32)
    t1 = np.asarray(t1, dtype=np.float32)
    wq = _bf16(np.asarray(Wq, dtype=np.float32))
    wk = _bf16(np.ascontiguousarray(Wkv[:, :C], dtype=np.float32))
    wv = _bf16(np.ascontiguousarray(Wkv[:, C:], dtype=np.float32))
    wp = _bf16(np.asarray(Wproj, dtype=np.float32))
    bp = _bf16(np.asarray(bproj, dtype=np.float32).reshape(1, C))
    in_maps = []
    for c in range(NCORES):
        b, qh = c // 2, c % 2
        in_maps.append({
            "t1T": _bf16(t1[b].T),
            "t2T": _bf16(t2[b].T[:, qh * Q:(qh + 1) * Q]),
            "wq": wq, "wk": wk, "wv": wv, "wp": wp, "bp": bp,
        })
    return in_maps


def kernel(t2_grad, t1, Wq, Wkv, Wproj, bproj, gamma, _trace=False,
           _use_fp32r=True):
    gamma = np.asarray(gamma)
    if float(np.abs(gamma).max()) != 0.0:
        # LAM block is only the identity for gamma == 0; fall back to a
        # host reference for the general case (not exercised by the
        # reference setup_inputs, which fixes gamma = 0).
        return _host_reference(t2_grad, t1, Wq, Wkv, Wproj, bproj, gamma)

    nc = _get_nc()
    in_maps = make_in_maps(t2_grad, t1, Wq, Wkv, Wproj, bproj)
    res = run_bass_kernel_spmd(nc, in_maps, list(range(NCORES)), trace=_trace)
    out = np.empty((B, N, C), dtype=np.float32)
    for c in range(NCORES):
        b, qh = c // 2, c % 2
        out[b, qh * Q:(qh + 1) * Q, :] = res.results[c]["out"]
    if _trace:
        _CACHE["last_result"] = res
    return out


def _host_reference(t2_grad, t1, Wq, Wkv, Wproj, bproj, gamma):
    t2 = np.asarray(t2_grad, dtype=np.float64)
    t1 = np.asarray(t1, dtype=np.float64)
    Wq = np.asarray(Wq, dtype=np.float64)
    Wkv = np.asarray(Wkv, dtype=np.float64)
    Wproj = np.asarray(Wproj, dtype=np.float64)
    bproj = np.asarray(bproj, dtype=np.float64)
    g = float(np.asarray(gamma).reshape(-1)[0])
    q = (t2 @ Wq).reshape(B, N, H, D).transpose(0, 2, 1, 3)
    kv = (t1 @ Wkv).reshape(B, N, 2, H, D).transpose(2, 0, 3, 1, 4)
    k, v = kv[0], kv[1]
    s = np.einsum('bhnd,bhmd->bhnm', q, k) * SCALE
    s = s - s.max(axis=-1, keepdims=True)
    p = np.exp(s)
    p /= p.sum(axis=-1, keepdims=True)
    x = np.einsum('bhnm,bhmd->bhnd', p, v)
    xp = x.transpose(0, 3, 1, 2).reshape(B, D, H * N)
    energy = xp @ xp.transpose(0, 2, 1)
    energy = energy - energy.max(axis=-1, keepdims=True)
    att = np.exp(energy)
    att /= att.sum(axis=-1, keepdims=True)
    lam_out = (att @ xp).reshape(B, D, H, N)
    lam_out = g * lam_out + xp.reshape(B, D, H, N)
    x = lam_out.transpose(0, 2, 3, 1)
    xo = x.transpose(0, 2, 1, 3).reshape(B, N, C) \
        + q.transpose(0, 2, 1, 3).reshape(B, N, C)
    return ((t2 + xo) @ Wproj + bproj).astype(np.float32)

